# revision 1
# baseline (speedup 1.0000x reference)
"""Trainium2 Bass kernel for DendralNeuron_Dynamic.

out[b,d] = min( min_f(x[b,f]-Wmin[d,f]), min_f(Wmax[d,f]-x[b,f]) )
  x: [1024, 784] f32, Wmin/Wmax: [512, 784] f32 -> out [1024, 512] f32

Strategy: data-parallel over batch B across 8 cores (128 rows each).
Per core, D=512 lives on partitions as 4 tiles of 128. Host precomputes
Wcat = concat(-Wmin, Wmax) [D, 2F]. On device, each x row b is
DMA-broadcast across the 128 partitions into the upper half of an SBUF
slot and the Scalar engine writes -x into the lower half, forming
xcat_b = [-x_b | x_b]. Then min(x-Wmin, Wmax-x) reduced over f collapses
to ONE fused DVE tensor_tensor_reduce per (b, d-tile):
  accum[d,1] = min_{2F}( Wcat[d,:] - xcat_b[:] )  -> out column b.
512 fused DVE instructions/core; raw Bass blocks with an explicit
semaphore pipeline (SP: DMA; ACT: negate; DVE: compute).
Output laid out [D, B_loc] per core; host transposes + concatenates.
"""

import numpy as np

B, F, D = 1024, 784, 512
F2 = 2 * F
NCORES = 8
B_LOC = B // NCORES  # 128
DT = D // 128  # 4 d-tiles
BIG = 3.0e38


def _get_subminreduce_op():
    """Register (once) a custom DVE op: out = in0 - in1,
    accum_out = min(s0, min_k out[k]). Runs via the per-NEFF custom-DVE
    table (the native TENSOR_TENSOR_REDUCE ISA opcode fails walrus
    codegen in this toolchain)."""
    from concourse.dve_ops import (
        OPS,
        CUSTOM_DVE_SPECS,
        DveOp,
        _CUSTOM_DVE_ROW_BASE,
        _SUB_OPCODE_FOR_NAME,
    )
    from concourse.dve_spec import C0, Spec, Src0, Src1, lower, minn
    from concourse.dve_uop import DveOpSpec

    name = "SUB_MIN_REDUCE_ANT_K"
    for op in OPS:
        if op.name == name:
            return op

    def _ref(in0, in1, c0, c1, c2):
        b = (in0.astype(np.float32) - in1).astype(np.float32)
        acc = np.minimum(b.reshape(b.shape[0], -1).min(axis=-1, keepdims=True), c0)
        return b, acc

    spec = Spec(body=Src0 - Src1, accum=minn, accum_init=C0, reference=_ref)
    row = _CUSTOM_DVE_ROW_BASE + len(OPS)
    assert row < 0x20, "custom-DVE row field overflow"
    _SUB_OPCODE_FOR_NAME[name] = row
    shas = {}
    for ver in ("v3", "v4"):
        tmp = DveOpSpec(name=name, opcode=row, uops=lower(spec, ver=ver), rd1_en=True)
        shas[ver] = tmp.sha(ver)
    op = DveOp(name, spec, subdim=False, uops_sha=shas)
    OPS.append(op)
    CUSTOM_DVE_SPECS[name] = spec
    return op


def build_nc(b_loc: int = B_LOC, xslots: int = 8, race_check: bool = False):
    """race_check=True: unique write-only scratch per TTR + race detector ON
    (small b_loc only) — validates the semaphore pipeline. Production uses
    shared scratch (write-only garbage, same-engine in-order => safe) with
    the detector off, since the detector rejects that benign WAW."""
    import concourse.bass as bass
    import concourse.mybir as mybir

    f32 = mybir.dt.float32
    sub = mybir.AluOpType.subtract
    amin = mybir.AluOpType.min
    copy_f = mybir.ActivationFunctionType.Copy

    smr_op = _get_subminreduce_op()
    nc = bass.Bass(detect_race_conditions=race_check)
    x_d = nc.declare_dram_parameter("x", [b_loc, F], f32, isOutput=False)
    wcat_d = nc.declare_dram_parameter("Wcat", [D, F2], f32, isOutput=False)
    out_d = nc.declare_dram_parameter("out", [D, b_loc], f32, isOutput=True)

    wt = [nc.alloc_sbuf_tensor(f"w{t}", [128, F2], f32) for t in range(DT)]
    xb = [nc.alloc_sbuf_tensor(f"xb{i}", [128, F2], f32) for i in range(xslots)]
    n_scr = b_loc * DT if race_check else 2
    scr = [nc.alloc_sbuf_tensor(f"scr{i}", [128, F2], f32) for i in range(n_scr)]
    osb = [nc.alloc_sbuf_tensor(f"osb{t}", [128, b_loc], f32) for t in range(DT)]

    wsem = nc.alloc_semaphore("wsem")
    xsems = [nc.alloc_semaphore(f"xsem{i}") for i in range(xslots)]
    asem = nc.alloc_semaphore("asem")
    vsem = nc.alloc_semaphore("vsem")
    osem = nc.alloc_semaphore("osem")

    with nc.Block() as block:

        @block.sync
        def _(sp):
            for t in range(DT):
                sp.dma_start(
                    out=wt[t][:, :], in_=wcat_d[t * 128:(t + 1) * 128, :]
                ).then_inc(wsem, 16)
            for b in range(b_loc):
                if b >= xslots:
                    # slot reuse: wait until DVE finished batch b-xslots
                    sp.wait_ge(vsem, DT * (b - xslots + 1))
                sp.dma_start(
                    out=xb[b % xslots][:, F:F2],
                    in_=x_d[b:b + 1, :].partition_broadcast(128),
                ).then_inc(xsems[b % xslots], 16)
            sp.wait_ge(vsem, DT * b_loc)
            for t in range(DT):
                sp.dma_start(
                    out=out_d[t * 128:(t + 1) * 128, :], in_=osb[t][:, :]
                ).then_inc(osem, 16)
            sp.wait_ge(osem, DT * 16)

        @block.scalar
        def _(act):
            for b in range(b_loc):
                act.wait_ge(xsems[b % xslots], 16 * (b // xslots + 1))
                s = b % xslots
                act.activation(
                    out=xb[s][:, 0:F], in_=xb[s][:, F:F2], func=copy_f, scale=-1.0
                ).then_inc(asem, 1)

        @block.vector
        def _(dve):
            dve.wait_ge(wsem, DT * 16)
            for b in range(b_loc):
                dve.wait_ge(asem, b + 1)
                s = b % xslots
                for t in range(DT):
                    si = (b * DT + t) if race_check else (t % 2)
                    dve.tensor_tensor(
                        out=scr[si][:, :],
                        in0=wt[t][:, :],
                        in1=xb[s][:, :],
                        op=sub,
                    )
                    red = dve.tensor_reduce(
                        out=osb[t][:, b:b + 1],
                        in_=scr[si][:, :],
                        axis=mybir.AxisListType.X,
                        op=amin,
                    )
                    if t == DT - 1:
                        red.then_inc(vsem, DT)

    return nc


def build_nc_pe(b_loc: int = B_LOC, xslots: int = 16, race_check: bool = False):
    """PE-assisted kernel: for each (b, d-tile) the Tensor engine computes
    psum[d, 0:2F] = Wcat[d,:] - xcat_b[:] via two accumulating matmuls
      mm1: I_128.T @ Wcat_t          (copies the bf16 weights into PSUM)
      mm2: ones2.T @ xmov_b          (adds [x|-x], split hi+lo for ~fp32
                                      accuracy; products by 1.0 are exact)
    and the Vector engine does the single fused pass that remains:
    a free-axis min-reduce of PSUM into the output column. DVE-bound at
    ~1 elem/cycle/lane, which is this problem's throughput floor.
    PSUM: two 4-bank buffers, ping-pong, chunks 512/512/512/32 so the
    valid 1568 columns are contiguous for the reduce."""
    import concourse.bass as bass
    import concourse.mybir as mybir

    f32 = mybir.dt.float32
    bf16 = mybir.dt.bfloat16
    amin = mybir.AluOpType.min

    nc = bass.Bass(detect_race_conditions=race_check)
    x2_d = nc.declare_dram_parameter("x2", [b_loc, 2, F2], bf16, isOutput=False)
    wcat_d = nc.declare_dram_parameter("Wcat", [D, F2], bf16, isOutput=False)
    id_d = nc.declare_dram_parameter("ident", [128, 128], bf16, isOutput=False)
    on_d = nc.declare_dram_parameter("ones2", [2, 128], bf16, isOutput=False)
    out_d = nc.declare_dram_parameter("out", [D, b_loc], f32, isOutput=True)

    wt = [nc.alloc_sbuf_tensor(f"w{t}", [128, F2], bf16) for t in range(DT)]
    xm = [nc.alloc_sbuf_tensor(f"xm{i}", [2, F2], bf16) for i in range(xslots)]
    id_sb = nc.alloc_sbuf_tensor("id_sb", [128, 128], bf16)
    on_sb = nc.alloc_sbuf_tensor("on_sb", [2, 128], bf16)
    osb = [nc.alloc_sbuf_tensor(f"osb{t}", [128, b_loc], f32) for t in range(DT)]
    pb = [nc.alloc_psum_tensor(f"pb{j}", [128, 2048], f32) for j in range(2)]

    wsem = nc.alloc_semaphore("wsem")
    xmsems = [nc.alloc_semaphore(f"xmsem{i}") for i in range(xslots)]
    psem = nc.alloc_semaphore("psem")
    vsem = nc.alloc_semaphore("vsem")
    osem = nc.alloc_semaphore("osem")

    CH = [(0, 512), (512, 512), (1024, 512), (1536, F2 - 1536)]

    with nc.Block() as block:

        @block.sync
        def _(sp):
            for t in range(DT):
                sp.dma_start(
                    out=wt[t][:, :], in_=wcat_d[t * 128:(t + 1) * 128, :]
                ).then_inc(wsem, 16)
            sp.dma_start(out=id_sb[:, :], in_=id_d[:, :]).then_inc(wsem, 16)
            sp.dma_start(out=on_sb[:, :], in_=on_d[:, :]).then_inc(wsem, 16)
            for b in range(b_loc):
                if b >= xslots:
                    sp.wait_ge(psem, DT * (b - xslots) + DT)
                sp.dma_start(
                    out=xm[b % xslots][:, :], in_=x2_d[b, :, :]
                ).then_inc(xmsems[b % xslots], 16)
            sp.wait_ge(vsem, DT * b_loc)
            for t in range(DT):
                sp.dma_start(
                    out=out_d[t * 128:(t + 1) * 128, :], in_=osb[t][:, :]
                ).then_inc(osem, 16)
            sp.wait_ge(osem, DT * 16)

        @block.tensor
        def _(pe):
            pe.wait_ge(wsem, 6 * 16)
            for b in range(b_loc):
                s = b % xslots
                pe.wait_ge(xmsems[s], 16 * (b // xslots + 1))
                for t in range(DT):
                    i = DT * b + t
                    j = i % 2
                    if i >= 2:
                        pe.wait_ge(vsem, i - 1)
                    for off, n in CH:
                        pe.matmul(
                            out=pb[j][:, off:off + n],
                            lhsT=id_sb[:, :],
                            rhs=wt[t][:, off:off + n],
                            start=True,
                            stop=False,
                        )
                    last = None
                    for off, n in CH:
                        last = pe.matmul(
                            out=pb[j][:, off:off + n],
                            lhsT=on_sb[:, :],
                            rhs=xm[s][:, off:off + n],
                            start=False,
                            stop=True,
                        )
                    last.then_inc(psem, 1)

        @block.vector
        def _(dve):
            for b in range(b_loc):
                for t in range(DT):
                    i = DT * b + t
                    dve.wait_ge(psem, i + 1)
                    dve.tensor_reduce(
                        out=osb[t][:, b:b + 1],
                        in_=pb[i % 2][:, 0:F2],
                        axis=mybir.AxisListType.X,
                        op=amin,
                    ).then_inc(vsem, 1)

    return nc


def build_nc_pe2(b_loc: int = B_LOC, xslots: int = 8, race_check: bool = False):
    """pe2: like build_nc_pe, but the idle Scalar engine copies each PSUM
    result tile into an 8-slot SBUF ring, and the DVE min-reduces FOUR
    tiles per instruction via a 3D access pattern [128, 4, 2F] -> [128, 4]
    (amortizes the per-instruction init 4x and reads SBUF instead of
    PSUM: 58 vs 120 init cycles). Output columns land in osb_all[:, 4b+t];
    the final DMA de-interleaves via a rearranged AP."""
    import concourse.bass as bass
    import concourse.mybir as mybir

    f32 = mybir.dt.float32
    bf16 = mybir.dt.bfloat16
    amin = mybir.AluOpType.min

    K_GRP = 4       # ops per DVE reduce group (= DT, one batch row b)
    NS = 8          # SBUF staging ring slots (2 groups)

    nc = bass.Bass(detect_race_conditions=race_check)
    x2_d = nc.declare_dram_parameter("x2", [b_loc, 2, F2], bf16, isOutput=False)
    wcat_d = nc.declare_dram_parameter("Wcat", [D, F2], bf16, isOutput=False)
    id_d = nc.declare_dram_parameter("ident", [128, 128], bf16, isOutput=False)
    on_d = nc.declare_dram_parameter("ones2", [2, 128], bf16, isOutput=False)
    out_d = nc.declare_dram_parameter("out", [D, b_loc], f32, isOutput=True)

    wt = [nc.alloc_sbuf_tensor(f"w{t}", [128, F2], bf16) for t in range(DT)]
    xm = [nc.alloc_sbuf_tensor(f"xm{i}", [2, F2], bf16) for i in range(xslots)]
    id_sb = nc.alloc_sbuf_tensor("id_sb", [128, 128], bf16)
    on_sb = nc.alloc_sbuf_tensor("on_sb", [2, 128], bf16)
    stg = nc.alloc_sbuf_tensor("stg", [128, NS, F2], f32)
    osb = nc.alloc_sbuf_tensor("osb", [128, DT, b_loc], f32)
    pb = [nc.alloc_psum_tensor(f"pb{j}", [128, 2048], f32) for j in range(2)]

    wsem = nc.alloc_semaphore("wsem")
    xmsems = [nc.alloc_semaphore(f"xmsem{i}") for i in range(xslots)]
    psem = nc.alloc_semaphore("psem")   # PE matmul groups done (per op)
    csem = nc.alloc_semaphore("csem")   # ACT copies done (per op)
    vsem = nc.alloc_semaphore("vsem")   # DVE ops done (per K_GRP group, +K_GRP)
    osem = nc.alloc_semaphore("osem")

    CH = [(0, 512), (512, 512), (1024, 512), (1536, F2 - 1536)]
    n_ops = b_loc * DT

    with nc.Block() as block:

        @block.sync
        def _(sp):
            for t in range(DT):
                sp.dma_start(
                    out=wt[t][:, :], in_=wcat_d[t * 128:(t + 1) * 128, :]
                ).then_inc(wsem, 16)
            sp.dma_start(out=id_sb[:, :], in_=id_d[:, :]).then_inc(wsem, 16)
            sp.dma_start(out=on_sb[:, :], in_=on_d[:, :]).then_inc(wsem, 16)
            for b in range(b_loc):
                if b >= xslots:
                    sp.wait_ge(psem, DT * (b - xslots) + DT)
                sp.dma_start(
                    out=xm[b % xslots][:, :], in_=x2_d[b, :, :]
                ).then_inc(xmsems[b % xslots], 16)
            sp.wait_ge(vsem, n_ops)
            for t in range(DT):
                sp.dma_start(
                    out=out_d[t * 128:(t + 1) * 128, :], in_=osb[:, t, :]
                ).then_inc(osem, 16)
            sp.wait_ge(osem, DT * 16)

        @block.tensor
        def _(pe):
            pe.wait_ge(wsem, 6 * 16)
            for b in range(b_loc):
                s = b % xslots
                pe.wait_ge(xmsems[s], 16 * (b // xslots + 1))
                for t in range(DT):
                    i = DT * b + t
                    j = i % 2
                    if i >= 2:
                        # psum buffer free once ACT copied op i-2
                        pe.wait_ge(csem, i - 1)
                    for off, n in CH:
                        pe.matmul(
                            out=pb[j][:, off:off + n],
                            lhsT=id_sb[:, :],
                            rhs=wt[t][:, off:off + n],
                            start=True,
                            stop=False,
                        )
                    last = None
                    for off, n in CH:
                        last = pe.matmul(
                            out=pb[j][:, off:off + n],
                            lhsT=on_sb[:, :],
                            rhs=xm[s][:, off:off + n],
                            start=False,
                            stop=True,
                        )
                    last.then_inc(psem, 1)

        @block.scalar
        def _(act):
            for i in range(n_ops):
                g = i // K_GRP
                if i % K_GRP == 0 and i >= NS:
                    # ring slots for this group were last used by group g-2
                    act.wait_ge(vsem, K_GRP * (g - 1))
                act.wait_ge(psem, i + 1)
                act.copy(out=stg[:, i % NS, :], in_=pb[i % 2][:, 0:F2]).then_inc(
                    csem, 1
                )

        @block.vector
        def _(dve):
            for g in range(n_ops // K_GRP):
                i0 = g * K_GRP
                dve.wait_ge(csem, i0 + K_GRP)
                half = (g % 2) * K_GRP
                dve.tensor_reduce(
                    out=osb[:, :, g],
                    in_=stg[:, half:half + K_GRP, :],
                    axis=mybir.AxisListType.X,
                    op=amin,
                ).then_inc(vsem, K_GRP)

    return nc


def build_nc_pe3(b_loc: int = B_LOC, xslots: int = 8, race_check: bool = False):
    """pe3: pe2 plus (a) per-tile weight gating (PE starts once wt[0] +
    ident/ones are resident instead of after all weight DMAs) and
    (b) K_GRP=8 DVE reduce groups spanning two batch rows, with a
    permuted 16-slot staging ring so page order matches the t-major
    output AP: ACT writes op (b,t) to slot 8*(g%2) + 2t + (b%2)."""
    import concourse.bass as bass
    import concourse.mybir as mybir

    f32 = mybir.dt.float32
    bf16 = mybir.dt.bfloat16
    amin = mybir.AluOpType.min

    K_GRP = 4
    NS = 8

    nc = bass.Bass(detect_race_conditions=race_check)
    x2_d = nc.declare_dram_parameter("x2", [b_loc, 2, F2], bf16, isOutput=False)
    wcat_d = nc.declare_dram_parameter("Wcat", [D, F2], bf16, isOutput=False)
    id_d = nc.declare_dram_parameter("ident", [128, 128], bf16, isOutput=False)
    on_d = nc.declare_dram_parameter("ones2", [2, 128], bf16, isOutput=False)
    out_d = nc.declare_dram_parameter("out", [D, b_loc], f32, isOutput=True)

    wt = [nc.alloc_sbuf_tensor(f"w{t}", [128, F2], bf16) for t in range(DT)]
    xm = [nc.alloc_sbuf_tensor(f"xm{i}", [2, F2], bf16) for i in range(xslots)]
    id_sb = nc.alloc_sbuf_tensor("id_sb", [128, 128], bf16)
    on_sb = nc.alloc_sbuf_tensor("on_sb", [2, 128], bf16)
    stg = nc.alloc_sbuf_tensor("stg", [128, NS, F2], f32)
    osb = nc.alloc_sbuf_tensor("osb", [128, DT, b_loc], f32)
    pb = [nc.alloc_psum_tensor(f"pb{j}", [128, 2048], f32) for j in range(2)]

    iosem = nc.alloc_semaphore("iosem")
    wtsems = [nc.alloc_semaphore(f"wtsem{t}") for t in range(DT)]
    xmsems = [nc.alloc_semaphore(f"xmsem{i}") for i in range(xslots)]
    psem = nc.alloc_semaphore("psem")
    csem = nc.alloc_semaphore("csem")
    vsem = nc.alloc_semaphore("vsem")
    osem = nc.alloc_semaphore("osem")

    CH = [(0, 512), (512, 512), (1024, 512), (1536, F2 - 1536)]
    n_ops = b_loc * DT

    def slot(i):
        return i % NS

    with nc.Block() as block:

        @block.sync
        def _(sp):
            sp.dma_start(out=id_sb[:, :], in_=id_d[:, :]).then_inc(iosem, 16)
            sp.dma_start(out=on_sb[:, :], in_=on_d[:, :]).then_inc(iosem, 16)
            # Interleave the first x rows between weight tiles so PE's
            # op (b=0,t=0) is not gated behind the whole 1.6MB weight train
            # (per-tile wtsems + per-slot xmsems make any order safe).
            sp.dma_start(
                out=wt[0][:, :], in_=wcat_d[0:128, :]
            ).then_inc(wtsems[0], 16)
            sp.dma_start(out=xm[0][:, :], in_=x2_d[0, :, :]).then_inc(xmsems[0], 16)
            for t in range(1, DT):
                sp.dma_start(
                    out=wt[t][:, :], in_=wcat_d[t * 128:(t + 1) * 128, :]
                ).then_inc(wtsems[t], 16)
            for b in range(1, b_loc):
                if b >= xslots:
                    sp.wait_ge(psem, DT * (b - xslots) + DT)
                sp.dma_start(
                    out=xm[b % xslots][:, :], in_=x2_d[b, :, :]
                ).then_inc(xmsems[b % xslots], 16)
            sp.wait_ge(vsem, n_ops)
            for t in range(DT):
                sp.dma_start(
                    out=out_d[t * 128:(t + 1) * 128, :], in_=osb[:, t, :]
                ).then_inc(osem, 16)
            sp.wait_ge(osem, DT * 16)

        @block.tensor
        def _(pe):
            pe.wait_ge(iosem, 32)
            for b in range(b_loc):
                s = b % xslots
                pe.wait_ge(xmsems[s], 16 * (b // xslots + 1))
                for t in range(DT):
                    i = DT * b + t
                    j = i % 2
                    if b == 0:
                        pe.wait_ge(wtsems[t], 16)
                    if i >= 2:
                        pe.wait_ge(csem, i - 1)
                    for off, n in CH:
                        pe.matmul(
                            out=pb[j][:, off:off + n],
                            lhsT=id_sb[:, :],
                            rhs=wt[t][:, off:off + n],
                            start=True,
                            stop=False,
                        )
                    last = None
                    for off, n in CH:
                        last = pe.matmul(
                            out=pb[j][:, off:off + n],
                            lhsT=on_sb[:, :],
                            rhs=xm[s][:, off:off + n],
                            start=False,
                            stop=True,
                        )
                    last.then_inc(psem, 1)

        # Tapered reduce groups: sizes 1,1,2 then 4s. The first DVE
        # reduce starts after ACT copy #0 instead of #3 (~4us less fill).
        sizes = [1, 1, 2] + [K_GRP] * ((n_ops - 4) // K_GRP)
        assert sum(sizes) == n_ops
        group_start = [0]
        for sz in sizes:
            group_start.append(group_start[-1] + sz)
        group_of_op = []
        for g, sz in enumerate(sizes):
            group_of_op += [g] * sz

        @block.scalar
        def _(act):
            for i in range(n_ops):
                if i >= NS and slot(i) == slot(i - NS):
                    gprev = group_of_op[i - NS]
                    act.wait_ge(vsem, group_start[gprev + 1])
                act.wait_ge(psem, i + 1)
                act.copy(out=stg[:, slot(i), :], in_=pb[i % 2][:, 0:F2]).then_inc(
                    csem, 1
                )

        @block.vector
        def _(dve):
            for g, sz in enumerate(sizes):
                i0 = group_start[g]
                dve.wait_ge(csem, i0 + sz)
                s0 = i0 % NS
                b0, t0 = i0 // DT, i0 % DT
                if sz == K_GRP:
                    out_ap = osb[:, :, b0]
                else:
                    out_ap = osb[:, t0:t0 + sz, b0]
                dve.tensor_reduce(
                    out=out_ap,
                    in_=stg[:, s0:s0 + sz, :],
                    axis=mybir.AxisListType.X,
                    op=amin,
                ).then_inc(vsem, sz)

    return nc


def kernel_pe3(**inputs) -> np.ndarray:
    from concourse.bass_utils import run_bass_kernel_spmd

    x2, wcat, ident, ones2 = _prep_pe(inputs)
    nc = build_nc_pe3()
    in_maps = [
        {
            "x2": x2[c * B_LOC:(c + 1) * B_LOC],
            "Wcat": wcat,
            "ident": ident,
            "ones2": ones2,
        }
        for c in range(NCORES)
    ]
    res = run_bass_kernel_spmd(nc, in_maps, core_ids=list(range(NCORES)))
    outs = [res.results[c]["out"] for c in range(NCORES)]
    return np.concatenate([o.T for o in outs], axis=0).astype(np.float32)


def kernel_pe2(**inputs) -> np.ndarray:
    from concourse.bass_utils import run_bass_kernel_spmd

    x2, wcat, ident, ones2 = _prep_pe(inputs)
    nc = build_nc_pe2()
    in_maps = [
        {
            "x2": x2[c * B_LOC:(c + 1) * B_LOC],
            "Wcat": wcat,
            "ident": ident,
            "ones2": ones2,
        }
        for c in range(NCORES)
    ]
    res = run_bass_kernel_spmd(nc, in_maps, core_ids=list(range(NCORES)))
    outs = [res.results[c]["out"] for c in range(NCORES)]
    return np.concatenate([o.T for o in outs], axis=0).astype(np.float32)


def _prep_pe(inputs):
    import ml_dtypes

    bf = ml_dtypes.bfloat16
    x = np.asarray(inputs["x"], dtype=np.float32)
    wmin = np.asarray(inputs["Wmin"], dtype=np.float32)
    wmax = np.asarray(inputs["Wmax"], dtype=np.float32)
    wcat = np.concatenate([-wmin, wmax], axis=1).astype(bf)  # [D, 2F]
    x_hi = x.astype(bf)
    x_lo = (x - x_hi.astype(np.float32)).astype(bf)
    x2 = np.empty((x.shape[0], 2, F2), dtype=bf)
    x2[:, 0, :F] = x_hi
    x2[:, 0, F:] = -x_hi
    x2[:, 1, :F] = x_lo
    x2[:, 1, F:] = -x_lo
    ident = np.eye(128, dtype=bf)
    ones2 = np.ones((2, 128), dtype=bf)
    return x2, np.ascontiguousarray(wcat), ident, ones2


def kernel_pe(**inputs) -> np.ndarray:
    from concourse.bass_utils import run_bass_kernel_spmd

    x2, wcat, ident, ones2 = _prep_pe(inputs)
    nc = build_nc_pe()
    in_maps = [
        {
            "x2": x2[c * B_LOC:(c + 1) * B_LOC],
            "Wcat": wcat,
            "ident": ident,
            "ones2": ones2,
        }
        for c in range(NCORES)
    ]
    res = run_bass_kernel_spmd(nc, in_maps, core_ids=list(range(NCORES)))
    outs = [res.results[c]["out"] for c in range(NCORES)]
    return np.concatenate([o.T for o in outs], axis=0).astype(np.float32)


def _prep(inputs):
    x = np.ascontiguousarray(np.asarray(inputs["x"], dtype=np.float32))
    wmin = np.asarray(inputs["Wmin"], dtype=np.float32)
    wmax = np.asarray(inputs["Wmax"], dtype=np.float32)
    wcat = np.ascontiguousarray(np.concatenate([-wmin, wmax], axis=1))  # [D, 2F]
    return x, wcat


def kernel_ttsub(**inputs) -> np.ndarray:
    from concourse.bass_utils import run_bass_kernel_spmd

    x, wcat = _prep(inputs)
    nc = build_nc()
    in_maps = [
        {"x": x[c * B_LOC:(c + 1) * B_LOC], "Wcat": wcat} for c in range(NCORES)
    ]
    res = run_bass_kernel_spmd(nc, in_maps, core_ids=list(range(NCORES)))
    outs = [res.results[c]["out"] for c in range(NCORES)]  # each [D, B_LOC]
    return np.concatenate([o.T for o in outs], axis=0).astype(np.float32)


def kernel(**inputs) -> np.ndarray:
    return kernel_pe3(**inputs)


def _get_submin_body_op():
    """Body-only variant (no accum) for compile bisection."""
    from concourse.dve_ops import (
        OPS,
        CUSTOM_DVE_SPECS,
        DveOp,
        _CUSTOM_DVE_ROW_BASE,
        _SUB_OPCODE_FOR_NAME,
    )
    from concourse.dve_spec import Spec, Src0, Src1, lower
    from concourse.dve_uop import DveOpSpec

    name = "SUB_BODY_ANT_K"
    for op in OPS:
        if op.name == name:
            return op
    spec = Spec(
        body=Src0 - Src1,
        reference=lambda in0, in1, c0, c1, c2: (in0.astype(np.float32) - in1),
    )
    row = _CUSTOM_DVE_ROW_BASE + len(OPS)
    assert row < 0x20
    _SUB_OPCODE_FOR_NAME[name] = row
    shas = {}
    for ver in ("v3", "v4"):
        tmp = DveOpSpec(name=name, opcode=row, uops=lower(spec, ver=ver), rd1_en=True)
        shas[ver] = tmp.sha(ver)
    op = DveOp(name, spec, subdim=False, uops_sha=shas)
    OPS.append(op)
    CUSTOM_DVE_SPECS[name] = spec
    return op

